# revision 20
# baseline (speedup 1.0000x reference)
"""Causal multi-head attention kernel for Trainium2 (Bass/Tile), 8 NeuronCores.

Problem: q,k,v [B=4, H=16, S=2048, d=64] fp32; out = softmax(mask(QK^T/sqrt(d))) @ V.

Sharding: 64 (b,h) head-slices, 8 per core (pure data/head parallel, no comms).

Per-core algorithm (per head):
  - Load q,k natively as [128, 16, 64] tiles; PE-transpose into qT,kT [64, 2048]
    strips (d on partitions).
  - Load v natively with an appended ones-column: v' [128, 16, 65]; the ones
    column makes the PV matmul also produce the softmax row-sums for free.
  - For each q-block b (512 wide), for each k-chunk c (128 wide, causal c <= 4b+3):
      sT[128k, 512q] = kT_chunk.T @ qT_block           (TensorE)
      pT = exp(0.125 * sT)                              (ScalarE, PSUM->SBUF)
      diagonal chunks: pT *= upper-tri 0/1 mask         (VectorE)
      oT[65, 512] += v'_chunk.T @ pT                    (TensorE, PSUM accum)
    No max-subtraction: scores ~ N(0,1), exp is safe in fp32.
  - Output: copy oT to SBUF, PE-transpose back to [q, d] layout, multiply by
    reciprocal of the sums column, DMA out.
"""

import os

import numpy as np

import concourse.bacc as bacc
import concourse.bass as bass
import concourse.mybir as mybir
from concourse.bass_utils import run_bass_kernel_spmd
from concourse.masks import make_identity, make_upper_triangular
from concourse.tile import TileContext

B, H, S, D = 4, 16, 2048, 64
NCORES = 8
HPC = (B * H) // NCORES  # heads per core = 8
QB = 512                 # q-block width (fp32 moving-operand max)
KC = 128                 # k-chunk width (psum partition max)
NQB = S // QB            # 4 q-blocks per head
NKC = S // KC            # 16 k-chunks per head
NT = S // 128            # 16 row-tiles per head

FP32 = mybir.dt.float32
FP32R = mybir.dt.float32r  # fp32 bits, single-pass PE matmul (vs 2x2 passes)


def build_program() -> bass.Bass:
    nc = bacc.Bacc(None, target_bir_lowering=False, debug=False)

    q_in = nc.declare_dram_parameter("q", [HPC, S, D], FP32R, isOutput=False)
    k_in = nc.declare_dram_parameter("k", [HPC, S, D], FP32R, isOutput=False)
    v_in = nc.declare_dram_parameter("v", [HPC, S, D], FP32R, isOutput=False)
    out_p = nc.declare_dram_parameter("out", [HPC, S, D], FP32, isOutput=True)

    with TileContext(nc) as tc:
        with (
            tc.tile_pool(name="consts", bufs=1) as consts,
            tc.tile_pool(name="inp", bufs=2) as inp,
            tc.tile_pool(name="strip", bufs=2) as strip,
            tc.tile_pool(name="ppool", bufs=4) as ppool,
            tc.tile_pool(name="osb", bufs=2) as osb,
            tc.tile_pool(name="res", bufs=2) as res,
            tc.tile_pool(name="tp_ps", bufs=2, space="PSUM") as tp_ps,
            tc.tile_pool(name="s_ps", bufs=4, space="PSUM") as s_ps,
            tc.tile_pool(name="o_ps", bufs=2, space="PSUM") as o_ps,
        ):
            ident = consts.tile([128, 128], FP32)
            make_identity(nc, ident)
            ident_r = consts.tile([128, 128], FP32R)
            nc.vector.tensor_copy(ident_r, ident)
            # tri[p, j] = 1.0 if j >= p else 0.0  (valid = at-or-above diagonal)
            tri_f32 = consts.tile([128, 128], FP32)
            make_upper_triangular(nc, tri_f32, val=1.0, diag=True)
            tri = consts.tile([128, 128], FP32R)
            nc.vector.tensor_copy(tri, tri_f32)
            ones_c = consts.tile([128, NKC], FP32)
            nc.vector.memset(ones_c, 1.0)

            # Heads are processed in pairs: the pair's qT/kT strips live on
            # partitions 0-63 (head A) and 64-127 (head B), so the two K=64
            # S^T matmuls run concurrently in distinct PE row-groups
            # (tile_position inferred from base_partition).
            for hp in range(HPC // 2):
                heads = (2 * hp, 2 * hp + 1)
                # ---- load inputs for this head pair ----
                # q/k of both heads interleave per row-tile: [128, t, {A,B}, 64],
                # so a [128, 128] slab transposes into the packed layout
                # (head A -> partitions 0-63, head B -> 64-127) in one shot.
                q2_sb = inp.tile([128, NT, 2, D], FP32R, tag="q2_sb")
                k2_sb = inp.tile([128, NT, 2, D], FP32R, tag="k2_sb")
                v_sbs = []
                for x, h in enumerate(heads):
                    nc.sync.dma_start(
                        out=q2_sb[:, :, x],
                        in_=q_in[h].rearrange("(t p) d -> p t d", p=128),
                    )
                    nc.sync.dma_start(
                        out=k2_sb[:, :, x],
                        in_=k_in[h].rearrange("(t p) d -> p t d", p=128),
                    )
                    v_sb = inp.tile([128, NKC, D + 1], FP32R, tag="v_sb", name=f"v{x}")
                    nc.sync.dma_start(
                        out=v_sb[:, :, 0:D],
                        in_=v_in[h].rearrange("(t p) d -> p t d", p=128),
                    )
                    nc.vector.tensor_copy(v_sb[:, :, D], ones_c)
                    v_sbs.append(v_sb)

                # ---- build packed qT2, kT2 [128, 2048] via PE transposes ----
                qT2 = strip.tile([128, S], FP32R, tag="qT2")
                kT2 = strip.tile([128, S], FP32R, tag="kT2")
                for dst, src in ((qT2, q2_sb), (kT2, k2_sb)):
                    for g in range(NT // 4):
                        tp = tp_ps.tile([128, 4, 128], FP32R, tag="tp")
                        for i in range(4):
                            nc.tensor.transpose(tp[:, i], src[:, 4 * g + i], ident_r)
                        nc.vector.tensor_copy(
                            dst[:, 512 * g : 512 * (g + 1)].rearrange(
                                "p (i f) -> p i f", i=4
                            ),
                            tp,
                        )

                # ---- attention main loop (both heads interleaved) ----
                for b in range(NQB):
                    oTs = [
                        o_ps.tile([D + 1, QB], FP32, tag="oT", name=f"oT{x}")
                        for x in range(2)
                    ]
                    nchunks = 4 * (b + 1)
                    for c in range(nchunks):
                        t = c - 4 * b  # >= 0 on diagonal chunks
                        j0 = 128 * t if t >= 0 else 0
                        sTs = []
                        for x in range(2):
                            sT = s_ps.tile([128, QB], FP32, tag="sT")
                            nc.tensor.matmul(
                                sT[:, j0:QB],
                                kT2[64 * x : 64 * (x + 1), KC * c : KC * (c + 1)],
                                qT2[64 * x : 64 * (x + 1), QB * b + j0 : QB * (b + 1)],
                                start=True,
                                stop=True,
                            )
                            sTs.append(sT)
                        for x in range(2):
                            pT = ppool.tile([128, QB], FP32R, tag="pT")
                            nc.scalar.activation(
                                pT[:, j0:QB],
                                sTs[x][:, j0:QB],
                                mybir.ActivationFunctionType.Exp,
                                scale=0.125,  # 1/sqrt(64)
                            )
                            if t >= 0:
                                nc.vector.tensor_mul(
                                    pT[:, j0 : j0 + 128], pT[:, j0 : j0 + 128], tri
                                )
                            nc.tensor.matmul(
                                oTs[x][:, j0:QB],
                                v_sbs[x][:, c],
                                pT[:, j0:QB],
                                start=(c == 0),
                                stop=(c == nchunks - 1),
                            )

                    # ---- normalize + transpose back + store ----
                    for x in range(2):
                        oT_sb = osb.tile([D + 1, QB], FP32, tag="oT_sb")
                        nc.vector.tensor_copy(oT_sb, oTs[x])
                        otr = tp_ps.tile([128, 4, D + 1], FP32, tag="tp")
                        for i in range(4):
                            nc.tensor.transpose(
                                otr[:, i],
                                oT_sb[:, 128 * i : 128 * (i + 1)],
                                ident[0 : D + 1, 0 : D + 1],
                            )
                        rec = res.tile([128, 4], FP32, tag="rec")
                        nc.vector.reciprocal(rec, otr[:, :, D])
                        ores = res.tile([128, 4, D], FP32, tag="ores")
                        for i in range(4):
                            nc.vector.tensor_scalar_mul(
                                ores[:, i], otr[:, i, 0:D], rec[:, i : i + 1]
                            )
                        nc.sync.dma_start(
                            out=out_p[heads[x], QB * b : QB * (b + 1), :].rearrange(
                                "(t p) d -> p t d", p=128
                            ),
                            in_=ores,
                        )
    nc.compile()
    return nc


_NC_CACHE = None
LAST_RESULT = None


def kernel(q: np.ndarray, k: np.ndarray, v: np.ndarray) -> np.ndarray:
    global _NC_CACHE, LAST_RESULT
    if _NC_CACHE is None:
        _NC_CACHE = build_program()
    nc = _NC_CACHE

    def shard(x):
        x = np.ascontiguousarray(np.asarray(x, dtype=np.float32)).reshape(B * H, S, D)
        return [np.ascontiguousarray(x[i * HPC : (i + 1) * HPC]) for i in range(NCORES)]

    qs, ks, vs = shard(q), shard(k), shard(v)
    in_maps = [{"q": qs[i], "k": ks[i], "v": vs[i]} for i in range(NCORES)]
    trace = bool(int(os.environ.get("KERNEL_TRACE", "0")))
    result = run_bass_kernel_spmd(
        nc, in_maps, core_ids=list(range(NCORES)), trace=trace
    )
    LAST_RESULT = result
    out = np.concatenate([r["out"] for r in result.results], axis=0)
    return out.reshape(B, H, S, D)


# revision 21
# speedup vs baseline: 1.1158x; 1.1158x over previous
"""Causal multi-head attention kernel for Trainium2 (Bass/Tile), 8 NeuronCores.

Problem: q,k,v [B=4, H=16, S=2048, d=64] fp32; out = softmax(causal(QK^T/sqrt(d))) @ V.

Sharding: 64 (b,h) head-slices, 8 per core (pure head parallel, no comms).

Per-core algorithm (per head):
  - Load q,k natively as [128, 16, 64] tiles; PE-transpose into qT,kT [64, 2048]
    strips (d on partitions).  Loads cast fp32 -> bf16 in-DMA (SWDGE) when
    USE_BF16, so matmuls run single-pass with fast weight load.
  - Load v natively with an appended ones-column: v' [128, 16, 65]; the ones
    column makes the PV matmul also produce the softmax row-sums for free.
  - For each q-block b (512 wide), for each k-chunk c (128 wide, causal):
      sT[128k, <=512q] = kT_chunk.T @ qT_block          (TensorE)
      pT = exp(0.125 * sT)                              (ScalarE, PSUM->SBUF)
      diagonal granule: pT *= upper-tri 0/1 mask        (VectorE)
      oT[65, 512] += v'_chunk.T @ pT                    (TensorE, PSUM accum)
    No max-subtraction: scores ~ N(0,1), exp is safe in fp32.
  - Output: copy oT to SBUF, PE-transpose back to [q, d] layout, scale rows by
    reciprocal of the sums column, DMA out.
"""

import os

import numpy as np

import concourse.bacc as bacc
import concourse.bass as bass
import concourse.mybir as mybir
from concourse.bass_utils import run_bass_kernel_spmd
from concourse.masks import make_identity, make_upper_triangular
from concourse.tile import TileContext

B, H, S, D = 4, 16, 2048, 64
NCORES = 8
HPC = (B * H) // NCORES  # heads per core = 8
QB = 512                 # q-block width (one PSUM bank of fp32)
KC = 128                 # k-chunk width (psum partition max)
NQB = S // QB            # 4 q-blocks per head
NKC = S // KC            # 16 k-chunks per head
NT = S // 128            # 16 row-tiles per head

FP32 = mybir.dt.float32
FP32R = mybir.dt.float32r  # fp32 bits, single-pass PE matmul
BF16 = mybir.dt.bfloat16

USE_BF16 = True  # matmul operands in bf16 (vs float32r)


def build_program() -> bass.Bass:
    nc = bacc.Bacc(None, target_bir_lowering=False, debug=False)
    mmdt = BF16 if USE_BF16 else FP32R

    q_in = nc.declare_dram_parameter("q", [HPC, S, D], FP32, isOutput=False)
    k_in = nc.declare_dram_parameter("k", [HPC, S, D], FP32, isOutput=False)
    v_in = nc.declare_dram_parameter("v", [HPC, S, D], FP32, isOutput=False)
    out_p = nc.declare_dram_parameter("out", [HPC, S, D], FP32, isOutput=True)

    def load_cast(dst, src):
        if USE_BF16:
            nc.gpsimd.dma_start(out=dst, in_=src)  # SWDGE casts fp32->bf16
        else:
            nc.sync.dma_start(out=dst, in_=src)

    with TileContext(nc) as tc:
        with (
            tc.tile_pool(name="consts", bufs=1) as consts,
            tc.tile_pool(name="inp", bufs=2) as inp,
            tc.tile_pool(name="strip", bufs=2) as strip,
            tc.tile_pool(name="ppool", bufs=4) as ppool,
            tc.tile_pool(name="osb", bufs=2) as osb,
            tc.tile_pool(name="res", bufs=2) as res,
            tc.tile_pool(name="tp_ps", bufs=2, space="PSUM") as tp_ps,
            tc.tile_pool(name="s_ps", bufs=4, space="PSUM") as s_ps,
            tc.tile_pool(name="o_ps", bufs=2, space="PSUM") as o_ps,
        ):
            ident = consts.tile([128, 128], FP32)
            make_identity(nc, ident)
            ident_m = consts.tile([128, 128], mmdt)
            nc.vector.tensor_copy(ident_m, ident)
            # tri[p, j] = 1.0 if j >= p else 0.0  (valid = at-or-above diagonal)
            tri_f32 = consts.tile([128, 128], FP32)
            make_upper_triangular(nc, tri_f32, val=1.0, diag=True)
            tri = consts.tile([128, 128], mmdt)
            nc.vector.tensor_copy(tri, tri_f32)
            ones_c = consts.tile([128, NKC], FP32)
            nc.vector.memset(ones_c, 1.0)

            for h in range(HPC):
                # ---- load inputs for this head ----
                q_sb = inp.tile([128, NT, D], mmdt, tag="q_sb")
                load_cast(q_sb, q_in[h].rearrange("(t p) d -> p t d", p=128))
                k_sb = inp.tile([128, NT, D], mmdt, tag="k_sb")
                load_cast(k_sb, k_in[h].rearrange("(t p) d -> p t d", p=128))
                v_sb = inp.tile([128, NKC, D + 1], mmdt, tag="v_sb")
                load_cast(v_sb[:, :, 0:D], v_in[h].rearrange("(t p) d -> p t d", p=128))
                nc.vector.tensor_copy(v_sb[:, :, D], ones_c)

                # ---- build qT, kT [64, 2048] strips via PE transposes ----
                qT = strip.tile([64, S], mmdt, tag="qT")
                kT = strip.tile([64, S], mmdt, tag="kT")
                for dst, src in ((qT, q_sb), (kT, k_sb)):
                    for g in range(NT // 4):
                        tp = tp_ps.tile([64, 4, 128], mmdt, tag="tp")
                        for i in range(4):
                            nc.tensor.transpose(tp[:, i], src[:, 4 * g + i], ident_m)
                        nc.vector.tensor_copy(
                            dst[:, 512 * g : 512 * (g + 1)].rearrange(
                                "p (i f) -> p i f", i=4
                            ),
                            tp,
                        )

                # ---- attention main loop ----
                for b in range(NQB):
                    oT = o_ps.tile([D + 1, QB], FP32)
                    nchunks = 4 * (b + 1)
                    for c in range(nchunks):
                        t = c - 4 * b  # >= 0 on diagonal chunks
                        j0 = 128 * t if t >= 0 else 0
                        sT = s_ps.tile([128, QB], FP32, tag="sT")
                        nc.tensor.matmul(
                            sT[:, j0:QB],
                            kT[:, KC * c : KC * (c + 1)],
                            qT[:, QB * b + j0 : QB * (b + 1)],
                            start=True,
                            stop=True,
                        )
                        pT = ppool.tile([128, QB], mmdt, tag="pT")
                        nc.scalar.activation(
                            pT[:, j0:QB],
                            sT[:, j0:QB],
                            mybir.ActivationFunctionType.Exp,
                            scale=0.125,  # 1/sqrt(64)
                        )
                        if t >= 0:
                            nc.vector.tensor_mul(
                                pT[:, j0 : j0 + 128], pT[:, j0 : j0 + 128], tri
                            )
                        nc.tensor.matmul(
                            oT[:, j0:QB],
                            v_sb[:, c],
                            pT[:, j0:QB],
                            start=(c == 0),
                            stop=(c == nchunks - 1),
                        )

                    # ---- normalize + transpose back + store ----
                    oT_sb = osb.tile([D + 1, QB], FP32)
                    nc.vector.tensor_copy(oT_sb, oT)
                    otr = tp_ps.tile([128, 4, D + 1], FP32, tag="tp")
                    for i in range(4):
                        nc.tensor.transpose(
                            otr[:, i],
                            oT_sb[:, 128 * i : 128 * (i + 1)],
                            ident[0 : D + 1, 0 : D + 1],
                        )
                    rec = res.tile([128, 4], FP32)
                    nc.vector.reciprocal(rec, otr[:, :, D])
                    ores = res.tile([128, 4, D], FP32)
                    for i in range(4):
                        nc.vector.tensor_scalar_mul(
                            ores[:, i], otr[:, i, 0:D], rec[:, i : i + 1]
                        )
                    nc.sync.dma_start(
                        out=out_p[h, QB * b : QB * (b + 1), :].rearrange(
                            "(t p) d -> p t d", p=128
                        ),
                        in_=ores,
                    )
    nc.compile()
    return nc


_NC_CACHE = None
LAST_RESULT = None


def kernel(q: np.ndarray, k: np.ndarray, v: np.ndarray) -> np.ndarray:
    global _NC_CACHE, LAST_RESULT
    if _NC_CACHE is None:
        _NC_CACHE = build_program()
    nc = _NC_CACHE

    def shard(x):
        x = np.ascontiguousarray(np.asarray(x, dtype=np.float32)).reshape(B * H, S, D)
        return [np.ascontiguousarray(x[i * HPC : (i + 1) * HPC]) for i in range(NCORES)]

    qs, ks, vs = shard(q), shard(k), shard(v)
    in_maps = [{"q": qs[i], "k": ks[i], "v": vs[i]} for i in range(NCORES)]
    trace = bool(int(os.environ.get("KERNEL_TRACE", "0")))
    result = run_bass_kernel_spmd(
        nc, in_maps, core_ids=list(range(NCORES)), trace=trace
    )
    LAST_RESULT = result
    out = np.concatenate([r["out"] for r in result.results], axis=0)
    return out.reshape(B, H, S, D)
